# revision 25
# baseline (speedup 1.0000x reference)
"""MoE-LoRA layer (nn_MoELoRALayer) as a Bass/Tile kernel for 8 Trainium2 cores.

Computation (per token n):
    logits = x @ W_router.T                    # [N, 8]
    combine = renorm(top2(softmax(logits)))    # [N, 8]
    h       = x @ A_cat.T                      # [N, 128]   (8 experts x rank 16)
    hw      = h * combine_expanded             # [N, 128]
    out     = x @ W_base.T + b + 2.0 * hw @ B_cat.T

Sharding: data-parallel over tokens (1024 per core), all weights replicated.
Matmul operands are bf16 (cast host-side), accumulating in fp32 PSUM; bias
add and the output stay fp32. Per-core PE work is ~2213 matmuls (~0.5 ms at
the 213 ns/matmul N=512 streaming floor).

Structure per core:
  phase 1: router logits.T and LoRA down-projection h.T for both 512-token
    blocks accumulate in PSUM off the incoming xt stream (router matmul
    first per K-tile so one semaphore wait covers both); the top-2 routing
    math runs token-major after small PE transposes, pipelined so the PE
    never queues behind the serial DVE/ACT chain; combine weights are
    expanded across each expert's 16 ranks with a tiny expansion matmul and
    folded into h.T.
  phase 2: per 512-wide output tile, the 8 token accumulators open with the
    LoRA up-projection (hw.T @ B_cat.T) and then accumulate the 32 K-tiles
    of the streamed W_base.T; a DVE pass adds the (partition-broadcast) bias
    while copying PSUM out, and results DMA straight back.

Host-side layout prep (part of sharding):
    xt   [32, 128, 1024] = x_shard.T, K-tile major (contraction on partitions)
    wt   [4096, 4096]    = W_base.T
    at   [128, 32, 128]  = A.transpose(2,0,1) packed per K-tile (j = e*16+r)
    bft  [128, 4096]     = 2.0 * B.transpose(0,2,1).reshape(128, 4096)
    pkb  [128, 384]      = packed W_router.T (per K-tile) + expansion matrix
    ident [128, 128]     = identity for PE transposes
"""

import numpy as np

import concourse.bacc as bacc
import concourse.bass as bass
import concourse.mybir as mybir
import concourse.tile as tile
from concourse.bass_utils import run_bass_kernel_spmd

N_CORES = 8
D_IN = 4096
D_OUT = 4096
N_EXP = 8
R = 16
J = N_EXP * R           # 128
SCALING = 2.0
TOK = 1024              # tokens per core
K_TILES = D_IN // 128   # 32
N_TILES = TOK // 128    # 8
O_TILES = D_OUT // 512  # 8
BLK = 512               # token block for phase 1
N_BLKS = TOK // BLK     # 2

F32 = mybir.dt.float32
BF16 = mybir.dt.bfloat16

_CACHE = {}


def _build_program(finalize=True):
    key = ("nc", finalize)
    if key in _CACHE:
        return _CACHE[key]

    nc = bacc.Bacc(trn_type="TRN2")

    xt_d = nc.dram_tensor("xt", [K_TILES, 128, TOK], BF16, kind="ExternalInput")
    wt_d = nc.dram_tensor("wt", [D_IN, D_OUT], BF16, kind="ExternalInput")
    at_d = nc.dram_tensor("at", [128, K_TILES, J], BF16, kind="ExternalInput")
    bft_d = nc.dram_tensor("bft", [J, D_OUT], BF16, kind="ExternalInput")
    bvec_d = nc.dram_tensor("bvec", [D_OUT], F32, kind="ExternalInput")
    pkb_d = nc.dram_tensor("pkb", [128, 384], BF16, kind="ExternalInput")
    id_d = nc.dram_tensor("ident", [128, 128], F32, kind="ExternalInput")
    out_d = nc.dram_tensor("out", [TOK, D_OUT], F32, kind="ExternalOutput")

    xt = xt_d[:]
    wt = wt_d[:]
    out_ap = out_d[:]

    mm = nc.tensor.matmul

    with tile.TileContext(nc) as tc:
        with (
            tc.tile_pool(name="xt_pool", bufs=8) as xt_pool,
            tc.tile_pool(name="res", bufs=1) as res,
            tc.tile_pool(name="wt_pool", bufs=16) as wt_pool,
            tc.tile_pool(name="out_pool", bufs=6) as out_pool,
            tc.tile_pool(name="rsm", bufs=2) as rsm,
            tc.tile_pool(name="rbig", bufs=2) as rbig,
            tc.tile_pool(name="ps", bufs=8, space="PSUM") as ps,
        ):
            # ---- resident loads: small tensors first so phase 1 can start
            # while the xt stream is still arriving ----
            pkr = res.tile([128, 384], BF16)
            nc.sync.dma_start(out=pkr, in_=pkb_d[:])
            pkf = res.tile([128, 128], F32)
            nc.sync.dma_start(out=pkf, in_=id_d[:])
            at_sb = res.tile([128, K_TILES, J], BF16)
            nc.sync.dma_start(out=at_sb, in_=at_d[:])
            wrt_sb = pkr[:, 0:256].rearrange("p (k e) -> p k e", e=N_EXP)
            emat_sb = pkr[0:N_EXP, 256:384]
            ident_sb = pkf
            hwt_sb = res.tile([J, TOK], BF16)

            # xt in 8 chunks of 4 K-tiles: DMA triggers cost ~600ns each on
            # the SP queue, so few big transfers beat many small ones; chunk
            # granularity still lets the phase-1 K-loop start after ~1MB.
            xts = []
            for g in range(8):
                t = xt_pool.tile([128, 4, TOK], BF16, tag="xt", name=f"xt_{g}")
                nc.sync.dma_start(out=t, in_=xt[g * 4:(g + 1) * 4].transpose([1, 0, 2]))
                for kk in range(4):
                    xts.append(t[:, kk, :])

            bft_sb = res.tile([J, D_OUT], BF16)
            nc.sync.dma_start(out=bft_sb, in_=bft_d[:])
            bias_sb = res.tile([128, D_OUT], F32)
            nc.gpsimd.dma_start(
                out=bias_sb, in_=bvec_d[:].partition_broadcast(128)
            )

            # bias arrives on a SWDGE queue; observe it on the DVE clock once.
            btch = rsm.tile([1, 1], F32, tag="btch")
            nc.vector.tensor_copy(out=btch, in_=bias_sb[0:1, 0:1])

            # ---- phase 1: router + LoRA down-projection ----
            # Both blocks' K-loops run back-to-back on the PE (they consume the
            # incoming xt stream in order); the serial routing tails follow and
            # overlap with the start of phase 2.
            prs, phs = [], []
            for b in range(N_BLKS):
                bsl = slice(b * BLK, (b + 1) * BLK)
                pr = ps.tile([128, 512], F32, tag="ps", name=f"pr_{b}")
                ph = ps.tile([128, 512], F32, tag="ps", name=f"ph_{b}")
                for k in range(K_TILES):
                    # router first: its wait covers xts[k] for the A-path mm
                    mm(pr[:N_EXP, :], wrt_sb[:, k, :], xts[k][:, bsl],
                       start=(k == 0), stop=(k == K_TILES - 1))
                    mm(ph, at_sb[:, k, :], xts[k][:, bsl],
                       start=(k == 0), stop=(k == K_TILES - 1))
                prs.append(pr)
                phs.append(ph)

            logits = []
            for b in range(N_BLKS):
                logits_sb = rbig.tile([N_EXP, BLK], F32, tag="lg", name=f"lg_{b}")
                nc.vector.tensor_copy(out=logits_sb, in_=prs[b][:N_EXP, :])
                logits.append(logits_sb)

            # all forward transposes first: the PE never queues behind the
            # serial DVE/ACT routing chain (FIFO head-of-line blocking)
            ltoks = {}
            for b in range(N_BLKS):
                for c in range(BLK // 128):
                    csl = slice(c * 128, (c + 1) * 128)
                    pt = ps.tile([128, 512], F32, tag="ps", name=f"pt_{b}_{c}")
                    nc.tensor.transpose(
                        out=pt[:, :N_EXP],
                        in_=logits[b][:, csl],
                        identity=ident_sb[:N_EXP, :N_EXP],
                    )
                    ltok = rsm.tile([128, N_EXP], F32, tag="lt",
                                    name=f"lt_{b}_{c}", bufs=8)
                    nc.vector.tensor_copy(out=ltok, in_=pt[:, :N_EXP])
                    ltoks[b, c] = ltok

            # top-2 renormalized softmax weights, exact algebra:
            #   m1 = max_e l; t = l - m1; m2 = max_e (t | top1 -> -inf)
            #   combine_e = [t >= m2] * exp(t) / (1 + exp(m2))
            combs = {}
            for b in range(N_BLKS):
                for c in range(BLK // 128):
                    ltok = ltoks[b, c]
                    m1 = rsm.tile([128, 1], F32, tag="m1")
                    nc.vector.tensor_reduce(
                        m1, ltok, axis=mybir.AxisListType.X, op=mybir.AluOpType.max
                    )
                    t = rsm.tile([128, N_EXP], F32, tag="t")
                    nc.vector.tensor_scalar(
                        out=t, in0=ltok, scalar1=m1, scalar2=None,
                        op0=mybir.AluOpType.subtract,
                    )
                    eq = rsm.tile([128, N_EXP], F32, tag="eq")
                    nc.vector.tensor_scalar(
                        out=eq, in0=t, scalar1=0.0, scalar2=None,
                        op0=mybir.AluOpType.is_ge,
                    )
                    msk = rsm.tile([128, N_EXP], F32, tag="msk")
                    nc.vector.scalar_tensor_tensor(
                        out=msk, in0=eq, scalar=-1e30, in1=t,
                        op0=mybir.AluOpType.mult, op1=mybir.AluOpType.add,
                    )
                    m2 = rsm.tile([128, 1], F32, tag="m2")
                    nc.vector.tensor_reduce(
                        m2, msk, axis=mybir.AxisListType.X, op=mybir.AluOpType.max
                    )
                    e2 = rsm.tile([128, 1], F32, tag="e2")
                    nc.scalar.activation(e2, m2, mybir.ActivationFunctionType.Exp)
                    den = rsm.tile([128, 1], F32, tag="den")
                    nc.vector.tensor_scalar_add(den, e2, 1.0)
                    rec = rsm.tile([128, 1], F32, tag="rec")
                    nc.vector.reciprocal(rec, den)
                    et = rsm.tile([128, N_EXP], F32, tag="et")
                    nc.scalar.activation(et, t, mybir.ActivationFunctionType.Exp)
                    ge = rsm.tile([128, N_EXP], F32, tag="ge")
                    nc.vector.tensor_scalar(
                        out=ge, in0=t, scalar1=m2, scalar2=None,
                        op0=mybir.AluOpType.is_ge,
                    )
                    w = rsm.tile([128, N_EXP], F32, tag="w")
                    nc.vector.tensor_tensor(
                        out=w, in0=et, in1=ge, op=mybir.AluOpType.mult
                    )
                    comb = rsm.tile([128, N_EXP], F32, tag="comb",
                                    name=f"comb_{b}_{c}", bufs=8)
                    nc.vector.tensor_scalar_mul(comb, w, rec)
                    combs[b, c] = comb

            for b in range(N_BLKS):
                bsl = slice(b * BLK, (b + 1) * BLK)
                combt_sb = rbig.tile([N_EXP, BLK], BF16, tag="ct", name=f"ct_{b}")
                for c in range(BLK // 128):
                    csl = slice(c * 128, (c + 1) * 128)
                    pc = ps.tile([128, 512], F32, tag="ps", name=f"pc_{b}_{c}")
                    nc.tensor.transpose(
                        out=pc[:N_EXP, :128], in_=combs[b, c], identity=ident_sb
                    )
                    nc.vector.tensor_copy(out=combt_sb[:, csl], in_=pc[:N_EXP, :128])

                # expand combine across the 16 ranks of each expert:
                # combine_expT[j, n] = combT[j//16, n]  via  emat.T @ combT
                pce = ps.tile([128, 512], F32, tag="ps", name=f"pce_{b}")
                mm(pce, emat_sb, combt_sb, start=True, stop=True)
                hsb = rbig.tile([128, BLK], F32, tag="hs", name=f"hs_{b}")
                nc.vector.tensor_copy(out=hsb, in_=phs[b])
                nc.vector.tensor_tensor(
                    out=hwt_sb[:, bsl], in0=hsb, in1=pce, op=mybir.AluOpType.mult
                )

            # ---- phase 2: LoRA up-projection + base GEMM + bias ----
            for o in range(O_TILES):
                osl = slice(o * 512, (o + 1) * 512)
                accs = [
                    ps.tile([128, 512], F32, tag="ps", name=f"acc_{o}_{n}")
                    for n in range(N_TILES)
                ]
                # open each accumulator with the expert contribution: its PSUM
                # slot wait (DVE release) coalesces with the hwt DVE wait.
                for n in range(N_TILES):
                    mm(accs[n], hwt_sb[:, n * 128:(n + 1) * 128],
                       bft_sb[:, osl], start=True, stop=False)
                # K-chunked, token-tile-inner: 8 consecutive matmuls per
                # PSUM bank instead of cycling all 8 banks every K-tile
                # (psum-queue depth-cycling causes PE micro-idles).
                KC = 8
                for kc in range(K_TILES // KC):
                    wts = []
                    for kk in range(KC):
                        k = kc * KC + kk
                        wtt = wt_pool.tile([128, 512], BF16, tag="wt",
                                           name=f"wt_{o}_{k}")
                        nc.scalar.dma_start(
                            out=wtt, in_=wt[k * 128:(k + 1) * 128, osl]
                        )
                        wts.append(wtt)
                    for n in range(N_TILES):
                        for kk in range(KC):
                            k = kc * KC + kk
                            mm(accs[n], xts[k][:, n * 128:(n + 1) * 128],
                               wts[kk], start=False,
                               stop=(k == K_TILES - 1))
                for n in range(N_TILES):
                    osb = out_pool.tile([128, 512], F32, tag="ob",
                                        name=f"ob_{o}_{n}")
                    nc.vector.tensor_tensor(
                        out=osb, in0=accs[n], in1=bias_sb[:, osl],
                        op=mybir.AluOpType.add,
                    )
                    nc.sync.dma_start(
                        out=out_ap[n * 128:(n + 1) * 128, osl], in_=osb
                    )
                    if o < O_TILES - 1:
                        # WAR closer: makes the DVE (not the outbound DMA
                        # queue) the releaser of this staging slot, so the
                        # next tile's bias-add needs no cross-queue DMA wait.
                        nc.vector.memset(osb[0:1, 0:1], 0.0)

    if finalize:
        nc.finalize()
    _CACHE[key] = nc
    return nc


def _prep_inputs(x, W_base, b_base, W_router, A, B):
    """Shard + lay out inputs for the 8 cores. Returns list of in_maps."""
    import ml_dtypes
    bf16 = ml_dtypes.bfloat16
    x = np.asarray(x)
    W_base = np.asarray(W_base)
    b_base = np.asarray(b_base)
    W_router = np.asarray(W_router)
    A = np.asarray(A)
    B = np.asarray(B)
    x_flat = np.ascontiguousarray(x, dtype=np.float32).reshape(-1, D_IN)
    wt = np.ascontiguousarray(W_base.T.astype(bf16))
    at = np.ascontiguousarray(
        A.astype(np.float32, copy=False)
        .transpose(2, 0, 1)
        .reshape(K_TILES, 128, J)
        .transpose(1, 0, 2)
        .astype(bf16)
    )
    wrt = (
        W_router.T.astype(np.float32, copy=False)
        .reshape(K_TILES, 128, N_EXP)
        .transpose(1, 0, 2)
    )
    bft = np.ascontiguousarray(
        (SCALING * B.astype(np.float32, copy=False).transpose(0, 2, 1)
         .reshape(J, D_OUT)).astype(bf16)
    )
    bvec = np.ascontiguousarray(b_base, dtype=np.float32)
    # packed bf16 residents: [:, :256] wrt, [:8, 256:384] emat
    pkb = np.zeros((128, 384), dtype=bf16)
    pkb[:, 0:256] = wrt.reshape(128, K_TILES * N_EXP).astype(bf16)
    pkb[0:N_EXP, 256:384] = np.repeat(
        np.eye(N_EXP, dtype=np.float32), R, axis=1
    ).astype(bf16)
    ident = np.eye(128, dtype=np.float32)

    in_maps = []
    for c in range(N_CORES):
        shard = x_flat[c * TOK:(c + 1) * TOK]          # [1024, 4096]
        xt = np.ascontiguousarray(shard.T.astype(bf16)).reshape(K_TILES, 128, TOK)
        in_maps.append({
            "xt": xt, "wt": wt, "at": at, "bft": bft, "bvec": bvec,
            "pkb": pkb, "ident": ident,
        })
    return in_maps


def _run(in_maps, trace=False, **kw):
    nc = _build_program()
    return run_bass_kernel_spmd(
        nc, in_maps, core_ids=list(range(N_CORES)), trace=trace, **kw
    )


def kernel(x, W_base, b_base, W_router, A, B):
    orig_shape = np.asarray(x).shape
    in_maps = _prep_inputs(x, W_base, b_base, W_router, A, B)
    res = _run(in_maps)
    shards = [res.results[c]["out"] for c in range(N_CORES)]
    out = np.concatenate(shards, axis=0)
    return out.reshape(*orig_shape[:-1], D_OUT).astype(np.float32, copy=False)


# revision 26
# speedup vs baseline: 1.0025x; 1.0025x over previous
"""MoE-LoRA layer (nn_MoELoRALayer) as a Bass/Tile kernel for 8 Trainium2 cores.

Computation (per token n):
    logits = x @ W_router.T                    # [N, 8]
    combine = renorm(top2(softmax(logits)))    # [N, 8]
    h       = x @ A_cat.T                      # [N, 128]   (8 experts x rank 16)
    hw      = h * combine_expanded             # [N, 128]
    out     = x @ W_base.T + b + 2.0 * hw @ B_cat.T

Sharding: data-parallel over tokens (1024 per core), all weights replicated.
Matmul operands are bf16 (cast host-side), accumulating in fp32 PSUM; bias
add and the output stay fp32. Per-core PE work is ~2213 matmuls (~0.5 ms at
the 213 ns/matmul N=512 streaming floor).

Structure per core:
  phase 1: router logits.T and LoRA down-projection h.T for both 512-token
    blocks accumulate in PSUM off the incoming xt stream (router matmul
    first per K-tile so one semaphore wait covers both); the top-2 routing
    math runs token-major after small PE transposes, pipelined so the PE
    never queues behind the serial DVE/ACT chain; combine weights are
    expanded across each expert's 16 ranks with a tiny expansion matmul and
    folded into h.T.
  phase 2: per 512-wide output tile, the 8 token accumulators open with the
    LoRA up-projection (hw.T @ B_cat.T) and then accumulate the 32 K-tiles
    of the streamed W_base.T; a DVE pass adds the (partition-broadcast) bias
    while copying PSUM out, and results DMA straight back.

Host-side layout prep (part of sharding):
    xt   [32, 128, 1024] = x_shard.T, K-tile major (contraction on partitions)
    wt   [4096, 4096]    = W_base.T
    at   [128, 32, 128]  = A.transpose(2,0,1) packed per K-tile (j = e*16+r)
    bft  [128, 4096]     = 2.0 * B.transpose(0,2,1).reshape(128, 4096)
    pkb  [128, 384]      = packed W_router.T (per K-tile) + expansion matrix
    ident [128, 128]     = identity for PE transposes
"""

import numpy as np

import concourse.bacc as bacc
import concourse.bass as bass
import concourse.mybir as mybir
import concourse.tile as tile
from concourse.bass_utils import run_bass_kernel_spmd

N_CORES = 8
D_IN = 4096
D_OUT = 4096
N_EXP = 8
R = 16
J = N_EXP * R           # 128
SCALING = 2.0
TOK = 1024              # tokens per core
K_TILES = D_IN // 128   # 32
N_TILES = TOK // 128    # 8
O_TILES = D_OUT // 512  # 8
BLK = 512               # token block for phase 1
N_BLKS = TOK // BLK     # 2

F32 = mybir.dt.float32
BF16 = mybir.dt.bfloat16

_CACHE = {}


def _build_program(finalize=True):
    key = ("nc", finalize)
    if key in _CACHE:
        return _CACHE[key]

    nc = bacc.Bacc(trn_type="TRN2")

    xt_d = nc.dram_tensor("xt", [K_TILES, 128, TOK], BF16, kind="ExternalInput")
    wt_d = nc.dram_tensor("wt", [D_IN, D_OUT], BF16, kind="ExternalInput")
    at_d = nc.dram_tensor("at", [128, K_TILES, J], BF16, kind="ExternalInput")
    bft_d = nc.dram_tensor("bft", [J, D_OUT], BF16, kind="ExternalInput")
    bvec_d = nc.dram_tensor("bvec", [D_OUT], F32, kind="ExternalInput")
    pkb_d = nc.dram_tensor("pkb", [128, 384], BF16, kind="ExternalInput")
    id_d = nc.dram_tensor("ident", [128, 128], F32, kind="ExternalInput")
    out_d = nc.dram_tensor("out", [TOK, D_OUT], F32, kind="ExternalOutput")

    xt = xt_d[:]
    wt = wt_d[:]
    out_ap = out_d[:]

    mm = nc.tensor.matmul

    with tile.TileContext(nc) as tc:
        with (
            tc.tile_pool(name="xt_pool", bufs=8) as xt_pool,
            tc.tile_pool(name="res", bufs=1) as res,
            tc.tile_pool(name="wt_pool", bufs=16) as wt_pool,
            tc.tile_pool(name="out_pool", bufs=6) as out_pool,
            tc.tile_pool(name="rsm", bufs=2) as rsm,
            tc.tile_pool(name="rbig", bufs=2) as rbig,
            tc.tile_pool(name="ps", bufs=8, space="PSUM") as ps,
        ):
            # ---- resident loads: small tensors first so phase 1 can start
            # while the xt stream is still arriving ----
            pkr = res.tile([128, 384], BF16)
            nc.sync.dma_start(out=pkr, in_=pkb_d[:])
            pkf = res.tile([128, 128], F32)
            nc.sync.dma_start(out=pkf, in_=id_d[:])
            at_sb = res.tile([128, K_TILES, J], BF16)
            nc.sync.dma_start(out=at_sb, in_=at_d[:])
            wrt_sb = pkr[:, 0:256].rearrange("p (k e) -> p k e", e=N_EXP)
            emat_sb = pkr[0:N_EXP, 256:384]
            ident_sb = pkf
            hwt_sb = res.tile([J, TOK], BF16)

            # xt in 8 chunks of 4 K-tiles: DMA triggers cost ~600ns each on
            # the SP queue, so few big transfers beat many small ones; chunk
            # granularity still lets the phase-1 K-loop start after ~1MB.
            xts = []
            for g in range(8):
                t = xt_pool.tile([128, 4, TOK], BF16, tag="xt", name=f"xt_{g}")
                nc.sync.dma_start(out=t, in_=xt[g * 4:(g + 1) * 4].transpose([1, 0, 2]))
                for kk in range(4):
                    xts.append(t[:, kk, :])

            bft_sb = res.tile([J, D_OUT], BF16)
            nc.sync.dma_start(out=bft_sb, in_=bft_d[:])
            bias_sb = res.tile([128, D_OUT], F32)
            nc.gpsimd.dma_start(
                out=bias_sb, in_=bvec_d[:].partition_broadcast(128)
            )

            # bias arrives on a SWDGE queue; observe it on the DVE clock once.
            btch = rsm.tile([1, 1], F32, tag="btch")
            nc.vector.tensor_copy(out=btch, in_=bias_sb[0:1, 0:1])

            # ---- phase 1: router + LoRA down-projection ----
            # Both blocks' K-loops run back-to-back on the PE (they consume the
            # incoming xt stream in order); the serial routing tails follow and
            # overlap with the start of phase 2.
            prs, phs, logits = [], [], []
            for b in range(N_BLKS):
                bsl = slice(b * BLK, (b + 1) * BLK)
                pr = ps.tile([128, 512], F32, tag="ps", name=f"pr_{b}")
                ph = ps.tile([128, 512], F32, tag="ps", name=f"ph_{b}")
                for k in range(K_TILES):
                    # router first: its wait covers xts[k] for the A-path mm
                    mm(pr[:N_EXP, :], wrt_sb[:, k, :], xts[k][:, bsl],
                       start=(k == 0), stop=(k == K_TILES - 1))
                    mm(ph, at_sb[:, k, :], xts[k][:, bsl],
                       start=(k == 0), stop=(k == K_TILES - 1))
                # copy logits out immediately: pr's bank frees early so
                # phase-2 accumulators can claim PSUM slots during the tails
                logits_sb = rbig.tile([N_EXP, BLK], F32, tag="lg", name=f"lg_{b}")
                nc.vector.tensor_copy(out=logits_sb, in_=pr[:N_EXP, :])
                logits.append(logits_sb)
                prs.append(pr)
                phs.append(ph)

            # all forward transposes first: the PE never queues behind the
            # serial DVE/ACT routing chain (FIFO head-of-line blocking)
            ltoks = {}
            for b in range(N_BLKS):
                for c in range(BLK // 128):
                    csl = slice(c * 128, (c + 1) * 128)
                    pt = ps.tile([128, 512], F32, tag="ps", name=f"pt_{b}_{c}")
                    nc.tensor.transpose(
                        out=pt[:, :N_EXP],
                        in_=logits[b][:, csl],
                        identity=ident_sb[:N_EXP, :N_EXP],
                    )
                    ltok = rsm.tile([128, N_EXP], F32, tag="lt",
                                    name=f"lt_{b}_{c}", bufs=8)
                    nc.vector.tensor_copy(out=ltok, in_=pt[:, :N_EXP])
                    ltoks[b, c] = ltok

            # top-2 renormalized softmax weights, exact algebra:
            #   m1 = max_e l; t = l - m1; m2 = max_e (t | top1 -> -inf)
            #   combine_e = [t >= m2] * exp(t) / (1 + exp(m2))
            combs = {}
            for b in range(N_BLKS):
                for c in range(BLK // 128):
                    ltok = ltoks[b, c]
                    m1 = rsm.tile([128, 1], F32, tag="m1")
                    nc.vector.tensor_reduce(
                        m1, ltok, axis=mybir.AxisListType.X, op=mybir.AluOpType.max
                    )
                    t = rsm.tile([128, N_EXP], F32, tag="t")
                    nc.vector.tensor_scalar(
                        out=t, in0=ltok, scalar1=m1, scalar2=None,
                        op0=mybir.AluOpType.subtract,
                    )
                    eq = rsm.tile([128, N_EXP], F32, tag="eq")
                    nc.vector.tensor_scalar(
                        out=eq, in0=t, scalar1=0.0, scalar2=None,
                        op0=mybir.AluOpType.is_ge,
                    )
                    msk = rsm.tile([128, N_EXP], F32, tag="msk")
                    nc.vector.scalar_tensor_tensor(
                        out=msk, in0=eq, scalar=-1e30, in1=t,
                        op0=mybir.AluOpType.mult, op1=mybir.AluOpType.add,
                    )
                    m2 = rsm.tile([128, 1], F32, tag="m2")
                    nc.vector.tensor_reduce(
                        m2, msk, axis=mybir.AxisListType.X, op=mybir.AluOpType.max
                    )
                    e2 = rsm.tile([128, 1], F32, tag="e2")
                    nc.scalar.activation(e2, m2, mybir.ActivationFunctionType.Exp)
                    den = rsm.tile([128, 1], F32, tag="den")
                    nc.vector.tensor_scalar_add(den, e2, 1.0)
                    rec = rsm.tile([128, 1], F32, tag="rec")
                    nc.vector.reciprocal(rec, den)
                    et = rsm.tile([128, N_EXP], F32, tag="et")
                    nc.scalar.activation(et, t, mybir.ActivationFunctionType.Exp)
                    ge = rsm.tile([128, N_EXP], F32, tag="ge")
                    nc.vector.tensor_scalar(
                        out=ge, in0=t, scalar1=m2, scalar2=None,
                        op0=mybir.AluOpType.is_ge,
                    )
                    w = rsm.tile([128, N_EXP], F32, tag="w")
                    nc.vector.tensor_tensor(
                        out=w, in0=et, in1=ge, op=mybir.AluOpType.mult
                    )
                    comb = rsm.tile([128, N_EXP], F32, tag="comb",
                                    name=f"comb_{b}_{c}", bufs=8)
                    nc.vector.tensor_scalar_mul(comb, w, rec)
                    combs[b, c] = comb

            for b in range(N_BLKS):
                bsl = slice(b * BLK, (b + 1) * BLK)
                combt_sb = rbig.tile([N_EXP, BLK], BF16, tag="ct", name=f"ct_{b}")
                for c in range(BLK // 128):
                    csl = slice(c * 128, (c + 1) * 128)
                    pc = ps.tile([128, 512], F32, tag="ps", name=f"pc_{b}_{c}")
                    nc.tensor.transpose(
                        out=pc[:N_EXP, :128], in_=combs[b, c], identity=ident_sb
                    )
                    nc.vector.tensor_copy(out=combt_sb[:, csl], in_=pc[:N_EXP, :128])

                # expand combine across the 16 ranks of each expert:
                # combine_expT[j, n] = combT[j//16, n]  via  emat.T @ combT
                pce = ps.tile([128, 512], F32, tag="ps", name=f"pce_{b}")
                mm(pce, emat_sb, combt_sb, start=True, stop=True)
                hsb = rbig.tile([128, BLK], F32, tag="hs", name=f"hs_{b}")
                nc.vector.tensor_copy(out=hsb, in_=phs[b])
                nc.vector.tensor_tensor(
                    out=hwt_sb[:, bsl], in0=hsb, in1=pce, op=mybir.AluOpType.mult
                )

            # ---- phase 2: LoRA up-projection + base GEMM + bias ----
            for o in range(O_TILES):
                osl = slice(o * 512, (o + 1) * 512)
                accs = [
                    ps.tile([128, 512], F32, tag="ps", name=f"acc_{o}_{n}")
                    for n in range(N_TILES)
                ]
                # open each accumulator with the expert contribution: its PSUM
                # slot wait (DVE release) coalesces with the hwt DVE wait.
                for n in range(N_TILES):
                    mm(accs[n], hwt_sb[:, n * 128:(n + 1) * 128],
                       bft_sb[:, osl], start=True, stop=False)
                # K-chunked, token-tile-inner: 8 consecutive matmuls per
                # PSUM bank instead of cycling all 8 banks every K-tile
                # (psum-queue depth-cycling causes PE micro-idles).
                KC = 8
                for kc in range(K_TILES // KC):
                    wts = []
                    for kk in range(KC):
                        k = kc * KC + kk
                        wtt = wt_pool.tile([128, 512], BF16, tag="wt",
                                           name=f"wt_{o}_{k}")
                        nc.scalar.dma_start(
                            out=wtt, in_=wt[k * 128:(k + 1) * 128, osl]
                        )
                        wts.append(wtt)
                    for n in range(N_TILES):
                        for kk in range(KC):
                            k = kc * KC + kk
                            mm(accs[n], xts[k][:, n * 128:(n + 1) * 128],
                               wts[kk], start=False,
                               stop=(k == K_TILES - 1))
                for n in range(N_TILES):
                    osb = out_pool.tile([128, 512], F32, tag="ob",
                                        name=f"ob_{o}_{n}")
                    nc.vector.tensor_tensor(
                        out=osb, in0=accs[n], in1=bias_sb[:, osl],
                        op=mybir.AluOpType.add,
                    )
                    nc.sync.dma_start(
                        out=out_ap[n * 128:(n + 1) * 128, osl], in_=osb
                    )
                    if o < O_TILES - 1:
                        # WAR closer: makes the DVE (not the outbound DMA
                        # queue) the releaser of this staging slot, so the
                        # next tile's bias-add needs no cross-queue DMA wait.
                        nc.vector.memset(osb[0:1, 0:1], 0.0)

    if finalize:
        nc.finalize()
    _CACHE[key] = nc
    return nc


def _prep_inputs(x, W_base, b_base, W_router, A, B):
    """Shard + lay out inputs for the 8 cores. Returns list of in_maps."""
    import ml_dtypes
    bf16 = ml_dtypes.bfloat16
    x = np.asarray(x)
    W_base = np.asarray(W_base)
    b_base = np.asarray(b_base)
    W_router = np.asarray(W_router)
    A = np.asarray(A)
    B = np.asarray(B)
    x_flat = np.ascontiguousarray(x, dtype=np.float32).reshape(-1, D_IN)
    wt = np.ascontiguousarray(W_base.T.astype(bf16))
    at = np.ascontiguousarray(
        A.astype(np.float32, copy=False)
        .transpose(2, 0, 1)
        .reshape(K_TILES, 128, J)
        .transpose(1, 0, 2)
        .astype(bf16)
    )
    wrt = (
        W_router.T.astype(np.float32, copy=False)
        .reshape(K_TILES, 128, N_EXP)
        .transpose(1, 0, 2)
    )
    bft = np.ascontiguousarray(
        (SCALING * B.astype(np.float32, copy=False).transpose(0, 2, 1)
         .reshape(J, D_OUT)).astype(bf16)
    )
    bvec = np.ascontiguousarray(b_base, dtype=np.float32)
    # packed bf16 residents: [:, :256] wrt, [:8, 256:384] emat
    pkb = np.zeros((128, 384), dtype=bf16)
    pkb[:, 0:256] = wrt.reshape(128, K_TILES * N_EXP).astype(bf16)
    pkb[0:N_EXP, 256:384] = np.repeat(
        np.eye(N_EXP, dtype=np.float32), R, axis=1
    ).astype(bf16)
    ident = np.eye(128, dtype=np.float32)

    in_maps = []
    for c in range(N_CORES):
        shard = x_flat[c * TOK:(c + 1) * TOK]          # [1024, 4096]
        xt = np.ascontiguousarray(shard.T.astype(bf16)).reshape(K_TILES, 128, TOK)
        in_maps.append({
            "xt": xt, "wt": wt, "at": at, "bft": bft, "bvec": bvec,
            "pkb": pkb, "ident": ident,
        })
    return in_maps


def _run(in_maps, trace=False, **kw):
    nc = _build_program()
    return run_bass_kernel_spmd(
        nc, in_maps, core_ids=list(range(N_CORES)), trace=trace, **kw
    )


def kernel(x, W_base, b_base, W_router, A, B):
    orig_shape = np.asarray(x).shape
    in_maps = _prep_inputs(x, W_base, b_base, W_router, A, B)
    res = _run(in_maps)
    shards = [res.results[c]["out"] for c in range(N_CORES)]
    out = np.concatenate(shards, axis=0)
    return out.reshape(*orig_shape[:-1], D_OUT).astype(np.float32, copy=False)
